# revision 1
# baseline (speedup 1.0000x reference)
"""Causal self-attention layer (LN + QKV + RoPE + GQA attention + proj) on 8 trn2 cores.

Sharding: sequence-parallel. 8 cores = 4 packed sequences x 2 query-halves.
Core c=(s,h) owns query rows [h*512, h*512+512) of sequence s and computes the
full K/V for its sequence locally (no collectives). Keys are permuted on the
host so each core's own query rows come first; attention is key-permutation
invariant given the host-built causal mask and per-key RoPE tables.

All matmuls run in bf16 with fp32 PSUM accumulation. Weights are pre-tiled on
the host so every weight DMA is one fully contiguous block.
"""

import os
import sys
import numpy as np

try:
    import concourse.bass as bass  # noqa: F401
except Exception:  # pragma: no cover
    for p in ("/opt/trn_rl_repo", "/root/.axon_site/_ro/trn_rl_repo"):
        if os.path.isdir(p) and p not in sys.path:
            sys.path.insert(0, p)

import ml_dtypes
import concourse.bass as bass
import concourse.tile as tile
from concourse import bacc, mybir
from concourse.bass_utils import run_bass_kernel_spmd

F32 = mybir.dt.float32
BF16 = mybir.dt.bfloat16

CFG_FULL = dict(H=4096, NQ=32, NKV=8, D=128, S=1024, B=4)
BASE = 10000.0
EPS = 1e-5

LAST_EXEC_NS = None


def _ceil_div(a, b):
    return (a + b - 1) // b


def _geom(cfg):
    H, NQ, NKV, D, S = cfg["H"], cfg["NQ"], cfg["NKV"], cfg["D"], cfg["S"]
    g = {}
    g["RQ"] = S // 2
    g["RK"] = S
    g["HT"] = H // 128
    g["NT_K"] = S // 128
    g["NT_Q"] = g["RQ"] // 128
    g["VC"] = NKV * D
    g["REP"] = NQ // NKV
    g["GQ"] = min(8, NQ)
    g["NGQ"] = NQ // g["GQ"]
    g["GK"] = min(4, NKV)
    g["NGK"] = NKV // g["GK"]
    g["VCH"] = min(512, g["VC"])
    g["NCV"] = g["VC"] // g["VCH"]
    g["gcols"] = 1024 if H % 1024 == 0 else H
    g["NGP"] = H // g["gcols"]
    return g


def build_bass(cfg):
    """Build the single-core SPMD program (identical across cores)."""
    H, NQ, NKV, D, S = cfg["H"], cfg["NQ"], cfg["NKV"], cfg["D"], cfg["S"]
    assert D == 128
    g = _geom(cfg)
    RQ, RK, HT, NT_K, NT_Q = g["RQ"], g["RK"], g["HT"], g["NT_K"], g["NT_Q"]
    VC, REP = g["VC"], g["REP"]
    GQ, NGQ, GK, NGK = g["GQ"], g["NGQ"], g["GK"], g["NGK"]
    VCH, NCV, gcols, NGP = g["VCH"], g["NCV"], g["gcols"], g["NGP"]
    assert HT >= NQ and RK >= RQ  # attnT reuses xnT's space

    nc = bacc.Bacc(None, target_bir_lowering=False)

    x_d = nc.dram_tensor("x", [RK, H], BF16, kind="ExternalInput")
    # weights pre-tiled on host: every [128, cols] slice is contiguous
    wq_d = nc.dram_tensor("wq", [HT, NGQ, 128, GQ * 128], BF16,
                          kind="ExternalInput")
    wk_d = nc.dram_tensor("wk", [HT, NGK, 128, GK * 128], BF16,
                          kind="ExternalInput")
    wv_d = nc.dram_tensor("wv", [HT, NCV, 128, VCH], BF16,
                          kind="ExternalInput")
    wp_d = nc.dram_tensor("wp", [NQ, NGP, 128, gcols], BF16,
                          kind="ExternalInput")
    bq_d = nc.dram_tensor("bq", [128, NQ], F32, kind="ExternalInput")
    bk_d = nc.dram_tensor("bk", [128, NKV], F32, kind="ExternalInput")
    bv_d = nc.dram_tensor("bv", [1, VC], F32, kind="ExternalInput")
    bp_d = nc.dram_tensor("bp", [1, H], F32, kind="ExternalInput")
    cq_d = nc.dram_tensor("cq", [64, RQ], F32, kind="ExternalInput")
    sq_d = nc.dram_tensor("sq", [64, RQ], F32, kind="ExternalInput")
    ck_d = nc.dram_tensor("ck", [64, RK], F32, kind="ExternalInput")
    sk_d = nc.dram_tensor("sk", [64, RK], F32, kind="ExternalInput")
    mask_d = nc.dram_tensor("mask", [RK, RQ], BF16, kind="ExternalInput")
    out_d = nc.dram_tensor("out", [RQ, H], F32, kind="ExternalOutput")

    nck = _ceil_div(RK, 512)   # rhs chunks over key tokens

    with tile.TileContext(nc) as tc:
        with (
            tc.tile_pool(name="const", bufs=1) as const,
            tc.tile_pool(name="wstream", bufs=3) as wstream,
            tc.tile_pool(name="dram", bufs=1, space="DRAM") as dram,
        ):
            # ---- constants ----
            cq_sb = const.tile([64, RQ], F32, tag="cq")
            sq_sb = const.tile([64, RQ], F32, tag="sq")
            ck_sb = const.tile([64, RK], F32, tag="ck")
            sk_sb = const.tile([64, RK], F32, tag="sk")
            nc.sync.dma_start(out=cq_sb[:], in_=cq_d[:])
            nc.sync.dma_start(out=sq_sb[:], in_=sq_d[:])
            nc.sync.dma_start(out=ck_sb[:], in_=ck_d[:])
            nc.sync.dma_start(out=sk_sb[:], in_=sk_d[:])
            bq_sb = const.tile([128, NQ], F32, tag="bq")
            bk_sb = const.tile([128, NKV], F32, tag="bk")
            nc.sync.dma_start(out=bq_sb[:], in_=bq_d[:])
            nc.sync.dma_start(out=bk_sb[:], in_=bk_d[:])
            bv_sb = const.tile([128, VC], F32, tag="bv")
            nc.gpsimd.dma_start(
                out=bv_sb[:],
                in_=bass.AP(tensor=bv_d, offset=0, ap=[[0, 128], [1, VC]]),
            )
            ones_col = const.tile([128, 1], BF16, tag="ones_col")
            nc.vector.memset(ones_col[:], 1.0)
            ones_row = const.tile([1, 128], F32, tag="ones_row")
            nc.vector.memset(ones_row[:], 1.0)
            eps_sb = const.tile([128, 1], F32, tag="eps")
            nc.vector.memset(eps_sb[:], EPS)

            # transpose-friendly scratch: per H-tile, rows are contiguous
            xn_dram = dram.tile([HT, RK, 128], BF16)

            with tc.tile_pool(name="xnt_pool", bufs=1) as xnt_pool:
                xnT = xnt_pool.tile([128, HT, RK], BF16, tag="xnT")

                # ---- phase 1: LayerNorm (ln_g/b folded into weights) ----
                n_sub = _ceil_div(H, 512)
                sub = H // n_sub
                assert sub * n_sub == H and sub <= 512
                with (
                    tc.tile_pool(name="ln", bufs=2) as ln_pool,
                    tc.tile_pool(name="stat", bufs=3) as stat,
                ):
                    for tt in range(NT_K):
                        xt = ln_pool.tile([128, H], BF16, tag="xt")
                        nc.sync.dma_start(
                            out=xt[:], in_=x_d[tt * 128:(tt + 1) * 128, :])
                        stats = stat.tile([128, n_sub, 6], F32, tag="stats")
                        xt3 = xt[:].rearrange("p (n f) -> p n f", f=sub)
                        for si in range(n_sub):
                            nc.vector.bn_stats(out=stats[:, si, :],
                                               in_=xt3[:, si, :])
                        mv = stat.tile([128, 2], F32, tag="mv")
                        nc.vector.bn_aggr(out=mv[:], in_=stats[:])
                        rstd = stat.tile([128, 1], F32, tag="rstd")
                        nc.scalar.activation(
                            out=rstd[:], in_=mv[:, 1:2],
                            func=mybir.ActivationFunctionType.Sqrt,
                            bias=eps_sb[:], scale=1.0,
                        )
                        nc.vector.reciprocal(out=rstd[:], in_=rstd[:])
                        xnt = ln_pool.tile([128, H], BF16, tag="xnt")
                        nc.vector.tensor_scalar(
                            out=xnt[:], in0=xt[:],
                            scalar1=mv[:, 0:1], scalar2=rstd[:],
                            op0=mybir.AluOpType.subtract,
                            op1=mybir.AluOpType.mult,
                        )
                        # scatter into [HT, tok, 128] layout (256B lines)
                        nc.sync.dma_start(
                            out=xn_dram[:, tt * 128:(tt + 1) * 128, :]
                                .rearrange("h t d -> t h d"),
                            in_=xnt[:].rearrange("t (h d) -> t h d", d=128),
                        )

                # ---- phase 2: transpose xn -> xnT (two DMAs per
                # H-tile so the first-half tokens unblock the Q phase early)
                for half in range(2):
                    h0 = half * (RK // 2)
                    h1 = h0 + RK // 2
                    for ht in range(HT):
                        nc.sync.dma_start_transpose(
                            out=xnT[:, ht, h0:h1], in_=xn_dram[ht, h0:h1, :])

                # ---- phase 3: QKV projections ----
                with tc.tile_pool(name="qkvout", bufs=1) as qkvout:
                    QT = qkvout.tile([128, NQ, RQ], BF16, tag="QT")
                    KT = qkvout.tile([128, NKV, RK], BF16, tag="KT")
                    Vn = qkvout.tile([128, NT_K, VC], BF16, tag="Vn")

                    with (
                        tc.tile_pool(name="rope", bufs=2) as rope,
                        tc.tile_pool(name="ropet", bufs=1) as ropet,
                    ):
                        def rope_evac(psum_ap, bias_col, lo, hi):
                            # evacuate psum halves; hi half realigned to
                            # partition 0 (walrus: two-SBUF-input ops need
                            # equal input base partitions, so realign here)
                            nc.scalar.activation(
                                out=lo[:], in_=psum_ap[0:64, :],
                                func=mybir.ActivationFunctionType.Identity,
                                bias=bias_col[0:64], scale=1.0,
                            )
                            nc.scalar.activation(
                                out=hi[:], in_=psum_ap[64:128, :],
                                func=mybir.ActivationFunctionType.Identity,
                                bias=bias_col[64:128], scale=1.0,
                            )

                        def rope_apply(dst, lo, hi, cos_sb, sin_sb, n):
                            # dst[0:64]   = lo*cos - hi*sin
                            # dst[64:128] = lo*sin + hi*cos
                            for c0 in range(0, n, 512):
                                c1 = min(c0 + 512, n)
                                w = c1 - c0
                                t1 = ropet.tile([64, 512], F32, tag="t1")
                                t2 = ropet.tile([64, 512], F32, tag="t2")
                                nc.vector.tensor_mul(
                                    t1[:, :w], hi[:, c0:c1], sin_sb[:, c0:c1])
                                nc.vector.tensor_mul(
                                    t2[:, :w], lo[:, c0:c1], cos_sb[:, c0:c1])
                                nc.vector.tensor_sub(
                                    dst[0:64, c0:c1], t2[:, :w], t1[:, :w])
                                t3 = ropet.tile([64, 512], F32, tag="t3")
                                t4 = ropet.tile([64, 512], F32, tag="t4")
                                nc.vector.tensor_mul(
                                    t3[:, :w], lo[:, c0:c1], sin_sb[:, c0:c1])
                                nc.vector.tensor_mul(
                                    t4[:, :w], hi[:, c0:c1], cos_sb[:, c0:c1])
                                nc.vector.tensor_add(
                                    dst[64:128, c0:c1], t4[:, :w], t3[:, :w])

                        # -- Q: QT[h] = wq[:,h].T @ xnT[:, :RQ] --
                        with tc.tile_pool(name="ps_q", bufs=GQ,
                                          space="PSUM") as ps_q:
                            for gidx in range(NGQ):
                                psq = [ps_q.tile([128, RQ], F32, tag="ps",
                                                 name=f"psq{gi}")
                                       for gi in range(GQ)]
                                for k in range(HT):
                                    wb = wstream.tile([128, GQ * 128], BF16,
                                                      tag="wq")
                                    nc.sync.dma_start(
                                        out=wb[:], in_=wq_d[k, gidx])
                                    for gi in range(GQ):
                                        nc.tensor.matmul(
                                            psq[gi][:],
                                            wb[:, gi * 128:(gi + 1) * 128],
                                            xnT[:, k, 0:RQ],
                                            start=(k == 0),
                                            stop=(k == HT - 1),
                                        )
                                for gi in range(GQ):
                                    h = gidx * GQ + gi
                                    qlo = rope.tile([64, RQ], F32, tag="qlo")
                                    qhi = rope.tile([64, RQ], F32, tag="qhi")
                                    rope_evac(psq[gi][:], bq_sb[:, h:h + 1],
                                              qlo, qhi)
                                    rope_apply(QT[:, h, :], qlo, qhi,
                                               cq_sb, sq_sb, RQ)

                        # -- K: KT[h] = wk[:,h].T @ xnT (transposed layout) --
                        with tc.tile_pool(name="ps_k", bufs=GK,
                                          space="PSUM") as ps_k:
                            for gidx in range(NGK):
                                psk = [ps_k.tile([128, RK], F32, tag="ps",
                                                 name=f"psk{gi}")
                                       for gi in range(GK)]
                                for k in range(HT):
                                    wb = wstream.tile([128, GK * 128], BF16,
                                                      tag="wk")
                                    nc.sync.dma_start(
                                        out=wb[:], in_=wk_d[k, gidx])
                                    for gi in range(GK):
                                        for ch in range(nck):
                                            c0 = ch * 512
                                            c1 = min(c0 + 512, RK)
                                            nc.tensor.matmul(
                                                psk[gi][:, c0:c1],
                                                wb[:, gi * 128:(gi + 1) * 128],
                                                xnT[:, k, c0:c1],
                                                start=(k == 0),
                                                stop=(k == HT - 1),
                                            )
                                for gi in range(GK):
                                    h = gidx * GK + gi
                                    klo = rope.tile([64, RK], F32, tag="klo")
                                    khi = rope.tile([64, RK], F32, tag="khi")
                                    rope_evac(psk[gi][:], bk_sb[:, h:h + 1],
                                              klo, khi)
                                    rope_apply(KT[:, h, :], klo, khi,
                                               ck_sb, sk_sb, RK)

                        # -- V: Vn[t] = xnT[t].T @ wv (natural layout) --
                        with tc.tile_pool(name="ps_v", bufs=NT_K,
                                          space="PSUM") as ps_v:
                            for vch in range(NCV):
                                c0 = vch * VCH
                                psv = [ps_v.tile([128, VCH], F32, tag="ps",
                                                 name=f"psv{ti}")
                                       for ti in range(NT_K)]
                                for k in range(HT):
                                    wb = wstream.tile([128, VCH], BF16,
                                                      tag="wv")
                                    nc.sync.dma_start(
                                        out=wb[:], in_=wv_d[k, vch])
                                    for tt in range(NT_K):
                                        nc.tensor.matmul(
                                            psv[tt][:],
                                            xnT[:, k, tt * 128:(tt + 1) * 128],
                                            wb[:],
                                            start=(k == 0),
                                            stop=(k == HT - 1),
                                        )
                                for tt in range(NT_K):
                                    nc.vector.scalar_tensor_tensor(
                                        out=Vn[:, tt, c0:c0 + VCH],
                                        in0=psv[tt][:], scalar=1.0,
                                        in1=bv_sb[:, c0:c0 + VCH],
                                        op0=mybir.AluOpType.mult,
                                        op1=mybir.AluOpType.add,
                                    )

                    # ---- phase 4: attention per q head ----
                    # attnT[h] lives in xnT's (now dead) space: xnT[:, h, 0:RQ]
                    with (
                        tc.tile_pool(name="att", bufs=2) as att,
                        tc.tile_pool(name="small", bufs=2) as small,
                        tc.tile_pool(name="msk", bufs=1) as msk,
                        tc.tile_pool(name="ps_att", bufs=2,
                                     space="PSUM") as ps_att,
                    ):
                        mask_sb = msk.tile([128, NT_K, RQ], BF16, tag="mask")
                        nc.sync.dma_start(
                            out=mask_sb[:],
                            in_=mask_d[:].rearrange("(t p) q -> p t q", p=128),
                        )
                        for h in range(NQ):
                            gkv = h // REP
                            et = att.tile([128, NT_K, RQ], BF16, tag="expT")
                            for kt in range(NT_K):
                                sps = ps_att.tile([128, RQ], F32, tag="s")
                                nc.tensor.matmul(
                                    sps[:],
                                    KT[:, gkv, kt * 128:(kt + 1) * 128],
                                    QT[:, h, :],
                                    start=True, stop=True,
                                )
                                nc.scalar.activation(
                                    out=et[:, kt, :], in_=sps[:],
                                    func=mybir.ActivationFunctionType.Exp,
                                )
                                nc.vector.tensor_mul(
                                    et[:, kt, :], et[:, kt, :],
                                    mask_sb[:, kt, :])
                            ops_ = ps_att.tile([128, RQ], F32, tag="o")
                            for kt in range(NT_K):
                                nc.tensor.matmul(
                                    ops_[:],
                                    Vn[:, kt, gkv * D:(gkv + 1) * D],
                                    et[:, kt, :],
                                    start=(kt == 0), stop=(kt == NT_K - 1),
                                )
                            dps = ps_att.tile([1, RQ], F32, tag="d")
                            for kt in range(NT_K):
                                nc.tensor.matmul(
                                    dps[:],
                                    ones_col[:],
                                    et[:, kt, :],
                                    start=(kt == 0), stop=(kt == NT_K - 1),
                                )
                            rec = small.tile([1, RQ], F32, tag="rec")
                            nc.vector.reciprocal(out=rec[:], in_=dps[:])
                            bcp = ps_att.tile([128, RQ], F32, tag="bc")
                            nc.tensor.matmul(
                                bcp[:], ones_row[:], rec[:],
                                start=True, stop=True)
                            # DVE can read only one PSUM operand: stage the
                            # reciprocal broadcast in SBUF via ACT
                            rbc = small.tile([128, RQ], F32, tag="rbc")
                            nc.scalar.copy(out=rbc[:], in_=bcp[:])
                            nc.vector.tensor_mul(
                                xnT[:, h, 0:RQ], ops_[:], rbc[:])

                    # ---- phase 5: out = attnT.T @ wp + bp ----
                    with (
                        tc.tile_pool(name="late", bufs=1) as late,
                        tc.tile_pool(name="outp", bufs=3) as outp,
                        tc.tile_pool(name="ps_c", bufs=4, space="PSUM") as ps_c,
                    ):
                        bp_sb = late.tile([128, H], F32, tag="bp")
                        nc.gpsimd.dma_start(
                            out=bp_sb[:],
                            in_=bass.AP(tensor=bp_d, offset=0,
                                        ap=[[0, 128], [1, H]]),
                        )
                        ncc = gcols // 512 if gcols % 512 == 0 else 1
                        ccw = gcols // ncc
                        for gp in range(NGP):
                            n0 = gp * gcols
                            psc = [ps_c.tile([128, gcols], F32, tag="c",
                                             name=f"psc{qt}")
                                   for qt in range(NT_Q)]
                            for k in range(NQ):
                                wb = wstream.tile([128, gcols], BF16, tag="wp")
                                nc.sync.dma_start(out=wb[:], in_=wp_d[k, gp])
                                for qt in range(NT_Q):
                                    for ch in range(ncc):
                                        c0, c1 = ch * ccw, (ch + 1) * ccw
                                        nc.tensor.matmul(
                                            psc[qt][:, c0:c1],
                                            xnT[:, k, qt * 128:(qt + 1) * 128],
                                            wb[:, c0:c1],
                                            start=(k == 0), stop=(k == NQ - 1),
                                        )
                            for qt in range(NT_Q):
                                ot = outp.tile([128, gcols], F32, tag="ot")
                                nc.vector.scalar_tensor_tensor(
                                    out=ot[:], in0=psc[qt][:], scalar=1.0,
                                    in1=bp_sb[:, n0:n0 + gcols],
                                    op0=mybir.AluOpType.mult,
                                    op1=mybir.AluOpType.add,
                                )
                                nc.sync.dma_start(
                                    out=out_d[qt * 128:(qt + 1) * 128,
                                              n0:n0 + gcols],
                                    in_=ot[:],
                                )

    nc.finalize()  # bacc register allocation; the pjrt path serializes as-is
    return nc


def prep_core_inputs(cfg, c, hidden, ln_g, ln_b, w_qkv, b_qkv, w_proj, b_proj,
                     shared):
    """Per-core input dict. `shared` caches the weight prep across cores."""
    H, NQ, NKV, D, S = cfg["H"], cfg["NQ"], cfg["NKV"], cfg["D"], cfg["S"]
    g = _geom(cfg)
    RQ = g["RQ"]
    if not shared:
        ln_g = np.asarray(ln_g, np.float32)
        ln_b = np.asarray(ln_b, np.float32)
        w_qkv = np.asarray(w_qkv, np.float32)
        b_qkv = np.asarray(b_qkv, np.float32)
        w_eff = ln_g[:, None] * w_qkv
        b_eff = b_qkv + ln_b @ w_qkv
        nqd, nkd = NQ * D, NKV * D
        HT, GQ, NGQ = g["HT"], g["GQ"], g["NGQ"]
        GK, NGK, VCH, NCV = g["GK"], g["NGK"], g["VCH"], g["NCV"]
        gcols, NGP = g["gcols"], g["NGP"]

        def tile_w(w, groups, gw):
            # [H, cols] -> [HT, groups, 128, gw] contiguous blocks
            return np.ascontiguousarray(
                w.reshape(HT, 128, groups, gw).transpose(0, 2, 1, 3)
            ).astype(ml_dtypes.bfloat16)

        shared["wq"] = tile_w(w_eff[:, :nqd], NGQ, GQ * 128)
        shared["wk"] = tile_w(w_eff[:, nqd:nqd + nkd], NGK, GK * 128)
        shared["wv"] = tile_w(w_eff[:, nqd + nkd:], NCV, VCH)
        wp = np.asarray(w_proj, np.float32)
        shared["wp"] = np.ascontiguousarray(
            wp.reshape(NQ, 128, NGP, gcols).transpose(0, 2, 1, 3)
        ).astype(ml_dtypes.bfloat16)
        shared["bq"] = np.ascontiguousarray(
            b_eff[:nqd].reshape(NQ, 128).T.astype(np.float32))
        shared["bk"] = np.ascontiguousarray(
            b_eff[nqd:nqd + nkd].reshape(NKV, 128).T.astype(np.float32))
        shared["bv"] = b_eff[nqd + nkd:].reshape(1, nkd).astype(np.float32)
        shared["bp"] = np.asarray(b_proj, np.float32).reshape(1, H)
        shared["inv_freq"] = (
            1.0 / (BASE ** (np.arange(0, D, 2, dtype=np.float32) / D)))

    s, h = c // 2, c % 2
    qpos = np.arange(h * RQ, h * RQ + RQ, dtype=np.float32)
    perm = np.concatenate([
        np.arange(h * RQ, h * RQ + RQ),
        np.arange((1 - h) * RQ, (1 - h) * RQ + RQ),
    ])
    x_c = np.ascontiguousarray(
        np.asarray(hidden, np.float32)[s * S:(s + 1) * S][perm]).astype(
            ml_dtypes.bfloat16)
    ivf = shared["inv_freq"][:, None]
    kpos = perm.astype(np.float32)[None, :]
    scale = float(D) ** -0.5
    ang_k = ivf * kpos
    ang_q = ivf * qpos[None, :]
    mask = (perm[:, None] <= (h * RQ + np.arange(RQ))[None, :])
    return dict(
        x=x_c,
        wq=shared["wq"], wk=shared["wk"], wv=shared["wv"], wp=shared["wp"],
        bq=shared["bq"], bk=shared["bk"], bv=shared["bv"], bp=shared["bp"],
        cq=(np.cos(ang_q) * scale).astype(np.float32),
        sq=(np.sin(ang_q) * scale).astype(np.float32),
        ck=np.cos(ang_k).astype(np.float32),
        sk=np.sin(ang_k).astype(np.float32),
        mask=mask.astype(ml_dtypes.bfloat16),
    )


_NC_CACHE = {}


def _get_nc(cfg_key, cfg):
    if cfg_key not in _NC_CACHE:
        _NC_CACHE[cfg_key] = build_bass(cfg)
    return _NC_CACHE[cfg_key]


def kernel(hidden_states, cu_seqlens, max_seqlen, ln_g, ln_b, w_qkv, b_qkv,
           w_proj, b_proj):
    global LAST_EXEC_NS
    cfg = CFG_FULL
    H, S, B = cfg["H"], cfg["S"], cfg["B"]
    T = B * S
    RQ = S // 2
    assert hidden_states.shape == (T, H)
    ncores = 2 * B

    shared = {}
    in_maps = [
        prep_core_inputs(cfg, c, hidden_states, ln_g, ln_b, w_qkv, b_qkv,
                         w_proj, b_proj, shared)
        for c in range(ncores)
    ]
    nc = _get_nc("full", cfg)
    res = run_bass_kernel_spmd(
        nc, in_maps, core_ids=list(range(ncores)),
        trace=bool(os.environ.get("BASS_TRACE")),
    )
    LAST_EXEC_NS = res.exec_time_ns
    out = np.empty((T, H), np.float32)
    for c in range(ncores):
        s, h = c // 2, c % 2
        r0 = s * S + h * RQ
        out[r0:r0 + RQ] = res.results[c]["out"]
    return out

